# revision 3
# baseline (speedup 1.0000x reference)
"""Trainium2 Bass kernel for fused causal multi-head attention
(qkv projection + causal softmax attention), B=2, T=4096, C=768, nH=12.

Sharding: 8 cores, core c -> batch b=c//4, head group g=c%4 (3 heads each).

Design (v4, lockstep batch-pairs):
  - bf16 projection, K=128 x 6 chunks, psum depth-3 (shared "s" tag).
  - attention in lockstep batch-pairs: round i takes the i-th k-tile step
    from the lo stream (PE rows 0-63) and hi stream (rows 64-127):
      * the two S^T matmuls land ADJACENT in the PE queue with disjoint
        row groups -> concurrent execution (2x matmul throughput);
      * both write one shared [128,1024] psum tile (different banks);
      * ONE wide exp op covers both streams' tiles (ScalarE true exp or
        VectorE exp2 bit-trick -> bf16, greedy-balanced);
      * PV matmuls are emitted with a 2-round delay so their pT operand
        is ready when the PE reaches them (no head-of-line blocking).
  - streams rebalanced to exactly 216 k-tile steps each
    (lo: h0 + h2 J in {0,1,2,3,7}; hi: h1 + h2 J in {4,5,6}).
  - causal masking: skipped/zeroed 128-blocks + gpsimd affine_select
    triangular boundary on the bf16 (int16-view) pT tile.
  - PV bf16 M=65 (ones column = softmax denominator), N-trimmed to the
    causally-valid q range; host divides by the denominator row.
  - V path: batched DMA-xbar transposes straight into v_aug layout.
  - PSUM: "s" tag [128,1024] x3 (6 banks, shared with projection) +
    po_lo + po_hi (2 banks).
"""
import sys
sys.path.insert(0, '/opt/trn_rl_repo')
import numpy as np

import os
import concourse.bass as bass
import concourse.tile as tile
from concourse import bacc, mybir
from concourse import bass_utils

FUSE = os.environ.get("FUSE", "1") == "1"
EXPDVE = os.environ.get("EXPDVE", "1") == "1"
DELAY = int(os.environ.get("DELAY", "3"))

B, T, C, NH = 2, 4096, 768, 12
HD = 64
NCORES = 8
NQ = T // 512
NKC = T // 128

BF = mybir.dt.bfloat16
I16 = mybir.dt.int16
F32 = mybir.dt.float32
AF = mybir.ActivationFunctionType
AL = mybir.AluOpType

LN2 = 0.6931471805599453
A = 1.0 / LN2
ACT_SCALE = 0.125 * LN2
ACT_BIAS = -2.0
DVE_MUL = 16.0
DVE_B = 128.0 * (127.0 - 2.0 / LN2) - 7.37
VST = 80

_CACHE = {}


def _build():
    if 'nc' in _CACHE:
        return _CACHE['nc']
    nc = bacc.Bacc("TRN2", target_bir_lowering=False, debug=False,
                   enable_asserts=True, num_devices=NCORES)
    xT_d = nc.dram_tensor("xT", [C, T], BF, kind="ExternalInput").ap()
    w_d = nc.dram_tensor("w", [128, 6 * 576], BF, kind="ExternalInput").ap()
    biasa_d = nc.dram_tensor("biasa", [128, 5], F32, kind="ExternalInput").ap()
    out_d = nc.dram_tensor("out", [195, T], BF, kind="ExternalOutput").ap()

    ew_ns = [0.0, 0.0]

    def ew_engine(fd, psum_src=True):
        ca = (172 + fd) / 1.2 if psum_src else (224 + fd) / 1.2
        cv = (120 + fd) / 0.96 if psum_src else (58 + fd) / 0.96
        if not EXPDVE or ew_ns[0] + ca <= ew_ns[1] + cv:
            ew_ns[0] += ca
            return 0
        ew_ns[1] += cv
        return 1

    with tile.TileContext(nc) as tc:
        with (
            tc.tile_pool(name="persist", bufs=1) as sb,
            tc.tile_pool(name="xn", bufs=18) as xpool,
            tc.tile_pool(name="ps", bufs=3, space="PSUM") as psp,
            tc.tile_pool(name="po", bufs=1, space="PSUM") as pop,
            tc.tile_pool(name="pT", bufs=4) as ptp,
            tc.tile_pool(name="oc", bufs=4) as ocp,
        ):
            xt_tiles = {}
            # x chunk-0 loads first so projection can start ASAP
            for c in range(6):
                t0 = xpool.tile([128, 512], BF, tag="xn", name=f"x0_{c}")
                dq = nc.sync if c % 2 == 0 else nc.scalar
                dq.dma_start(t0[:], xT_d[128 * c:128 * (c + 1), 0:512])
                xt_tiles[(0, c)] = t0
            w_sb = sb.tile([128, 6 * 576], BF, name="w")
            nc.scalar.dma_start(w_sb[:], w_d[:])
            biasa_sb = sb.tile([128, 5], F32, name="biasa")
            nc.sync.dma_start(biasa_sb[:], biasa_d[:])
            ebias = sb.tile([128, 1], F32, name="ebias")
            nc.vector.memset(ebias[:], ACT_BIAS)

            Qs = sb.tile([128, T], BF, name="Qs")
            Ks = sb.tile([128, T], BF, name="Ks")
            Q2d = sb.tile([128, T], BF, name="Q2d")
            K2d = sb.tile([128, T], BF, name="K2d")
            Q2h = sb.tile([128, T], BF, name="Q2h")
            K2h = sb.tile([128, T], BF, name="K2h")
            vsrc = [sb.tile([128, T], BF, name=f"vsrc{h}") for h in range(3)]
            VSRC_LO = [64, 64, 0]
            v_aug = [sb.tile([128, NKC * VST], BF, name=f"vaug{h}")
                     for h in range(3)]
            for h in range(3):
                ones_ap = v_aug[h][:].rearrange("p (k m) -> p k m", k=NKC)
                nc.gpsimd.memset(ones_ap[:, :, 64:65], 1.0)

            w3 = w_sb[:].rearrange("p (c m) -> p c m", c=6)
            MW = [128, 128, 128, 128, 64]

            def emit_merge(n, m, pj):
                sl = slice(512 * n, 512 * (n + 1))

                def ts(dst, src, brow, mult):
                    if ew_engine(512) == 0:
                        nc.scalar.activation(dst, src, AF.Identity,
                                             bias=brow, scale=mult)
                    else:
                        nc.vector.tensor_scalar(out=dst, in0=src, scalar1=mult,
                                                scalar2=brow, op0=AL.mult,
                                                op1=AL.add)

                if m == 0:
                    ts(Qs[:, sl], pj[:, :], biasa_sb[:, 0:1], A)
                elif m == 1:
                    ts(Ks[:, sl], pj[:, :], biasa_sb[:, 1:2], 1.0)
                elif m == 2:
                    ts(Q2d[0:64, sl], pj[0:64, :], biasa_sb[0:64, 2:3], A)
                    ts(vsrc[0][64:128, sl], pj[64:128, :],
                       biasa_sb[64:128, 2:3], 1.0)
                    nc.scalar.dma_start(Q2h[64:128, sl], Q2d[0:64, sl])
                elif m == 3:
                    ts(K2d[0:64, sl], pj[0:64, :], biasa_sb[0:64, 3:4], 1.0)
                    ts(vsrc[1][64:128, sl], pj[64:128, :],
                       biasa_sb[64:128, 3:4], 1.0)
                    nc.scalar.dma_start(K2h[64:128, sl], K2d[0:64, sl])
                else:
                    ts(vsrc[2][0:64, sl], pj[0:64, :], biasa_sb[0:64, 4:5], 1.0)

            def emit_vpath(n):
                if n % 2 == 0:
                    return
                kc0 = 4 * (n - 1)
                for h in range(3):
                    lo = VSRC_LO[h]
                    dq = nc.sync if h % 2 == 0 else nc.scalar
                    out3 = v_aug[h][:, VST * kc0:VST * (kc0 + 8)].rearrange(
                        "p (k m) -> p k m", k=8)[:, :, 0:64]
                    dq.dma_start_transpose(
                        out3, vsrc[h][lo:lo + 64, 128 * kc0:128 * (kc0 + 8)])

            def emit_x_load(n):
                for c in range(6):
                    t1 = xpool.tile([128, 512], BF, tag="xn", name=f"x{n}_{c}")
                    dq = nc.sync if c % 2 == 0 else nc.scalar
                    dq.dma_start(
                        t1[:], xT_d[128 * c:128 * (c + 1),
                                    512 * n:512 * (n + 1)])
                    xt_tiles[(n, c)] = t1

            def emit_proj_chunk(n):
                if n + 2 < NQ:
                    emit_x_load(n + 2)
                xn = [xt_tiles[(n, c)] for c in range(6)]
                for m in range(5):
                    mw = MW[m]
                    pj = psp.tile([128, 1024], F32, tag="s",
                                  name=f"pj{n}_{m}", bufs=3)
                    for c in range(6):
                        nc.tensor.matmul(
                            pj[:mw, 0:512],
                            lhsT=w3[:, c, 128 * m:128 * m + mw],
                            rhs=xn[c][:],
                            start=(c == 0), stop=(c == 5))
                    emit_merge(n, m, pj[:, 0:512])
                emit_vpath(n)

            # ---------------- attention: lockstep batch-pairs ----------------
            def qk_tiles(h, qlo):
                if h in (0, 1):
                    return (Ks, Qs)
                return (K2d, Q2d) if qlo == 0 else (K2h, Q2h)

            h2_lo = (0, 1, 2, 3, 7)
            lo_units = sorted([(J, 0) for J in range(NQ)] +
                              [(J, 2) for J in h2_lo])
            hi_units = sorted([(J, 1) for J in range(NQ)] +
                              [(J, 2) for J in range(NQ) if J not in h2_lo])

            def make_steps(units):
                steps = []
                for (J, h) in units:
                    nK = 4 * (J + 1)
                    for kc in range(nK):
                        steps.append((h, J, kc, nK))
                return steps

            lo_steps = make_steps(lo_units)
            hi_steps = make_steps(hi_units)
            assert len(lo_steps) == len(hi_steps) == 216, \
                (len(lo_steps), len(hi_steps))

            po_cur = {}
            pend = {'lo': [], 'hi': []}
            QLO = {'lo': 0, 'hi': 64}

            def emit_pv(s, e):
                (pTt, half, h, J, kc, nK) = e
                if kc == 0:
                    po_cur[s] = pop.tile([65, 512], F32, tag=f"po_{s}",
                                         name=f"po_{s}_{h}_{J}", bufs=1)
                po = po_cur[s]
                ci = max(0, 128 * (kc - 4 * J))
                nc.tensor.matmul(
                    po[:, ci:512],
                    lhsT=v_aug[h][:, VST * kc:VST * kc + 65],
                    rhs=pTt[:, 512 * half + ci:512 * (half + 1)].bitcast(BF),
                    start=(kc == 0), stop=(kc == nK - 1))
                if kc == nK - 1:
                    oc = ocp.tile([65, 512], BF, tag="oc",
                                  name=f"oc_{s}_{h}_{J}")
                    if ew_engine(512) == 0:
                        nc.scalar.copy(oc[:], po[:])
                    else:
                        nc.vector.tensor_copy(oc[:], po[:])
                    dq = nc.sync if (h + J) % 2 == 0 else nc.scalar
                    dq.dma_start(
                        out_d[65 * h:65 * (h + 1), 512 * J:512 * (J + 1)],
                        oc[:])

            def emit_pair(i):
                lo_h, lo_J, lo_kc, lo_nK = lo_steps[i]
                hi_h, hi_J, hi_kc, hi_nK = hi_steps[i]
                ps = psp.tile([128, 1024], F32, tag="s", name=f"ps{i}", bufs=3)
                for s, (h, J, kc) in (('lo', (lo_h, lo_J, lo_kc)),
                                      ('hi', (hi_h, hi_J, hi_kc))):
                    qlo = QLO[s]
                    Kt, Qt = qk_tiles(h, qlo)
                    half = 0 if s == 'lo' else 1
                    cc = max(0, 128 * (kc - 4 * J))
                    nc.tensor.matmul(
                        ps[:, 512 * half + cc:512 * (half + 1)],
                        lhsT=Kt[qlo:qlo + 64, 128 * kc:128 * (kc + 1)],
                        rhs=Qt[qlo:qlo + 64, 512 * J + cc:512 * (J + 1)],
                        start=True, stop=True)
                pT = ptp.tile([128, 1024], I16, tag="pt", name=f"pt{i}",
                              bufs=DELAY + 2)

                def expop(col0, col1):
                    if col1 <= col0:
                        return
                    dst = pT[:, col0:col1]
                    if ew_engine(col1 - col0) == 0:
                        nc.scalar.activation(dst.bitcast(BF), ps[:, col0:col1],
                                             AF.Exp, bias=ebias[:, 0:1],
                                             scale=ACT_SCALE)
                    else:
                        nc.vector.tensor_scalar(
                            out=dst, in0=ps[:, col0:col1], scalar1=DVE_MUL,
                            scalar2=DVE_B, op0=AL.mult, op1=AL.add)

                d_lo = lo_kc - 4 * lo_J
                d_hi = hi_kc - 4 * hi_J
                c_lo = 128 * d_lo if d_lo >= 0 else 0
                c_hi = 128 * d_hi if d_hi >= 0 else 0
                if c_lo == 0 and c_hi == 0:
                    expop(0, 1024)
                else:
                    expop(c_lo, 512)
                    expop(512 + c_hi, 1024)
                for half, dd, cc in ((0, d_lo, c_lo), (1, d_hi, c_hi)):
                    if dd >= 0:
                        blk = slice(512 * half + cc, 512 * half + cc + 128)
                        nc.gpsimd.affine_select(
                            out=pT[:, blk], in_=pT[:, blk],
                            compare_op=AL.is_ge, fill=0,
                            base=0, channel_multiplier=-1, pattern=[[1, 128]])
                pend['lo'].append((pT, 0, lo_h, lo_J, lo_kc, lo_nK))
                pend['hi'].append((pT, 1, hi_h, hi_J, hi_kc, hi_nK))
                for s in ('lo', 'hi'):
                    if len(pend[s]) > DELAY:
                        emit_pv(s, pend[s].pop(0))

            emit_x_load(1)
            i = 0
            for n in range(NQ):
                emit_proj_chunk(n)
                if FUSE:
                    gate = n - 1
                    while i < 216 and max(lo_steps[i][1],
                                          hi_steps[i][1]) <= gate:
                        emit_pair(i)
                        i += 1
            while i < 216:
                emit_pair(i)
                i += 1
            for s in ('lo', 'hi'):
                while pend[s]:
                    emit_pv(s, pend[s].pop(0))

    nc.compile()
    _CACHE['nc'] = nc
    return nc


def _prep_inputs(x, w_qkv, b_qkv):
    import ml_dtypes
    bf = ml_dtypes.bfloat16
    x = np.asarray(x, dtype=np.float32)
    w_qkv = np.asarray(w_qkv, dtype=np.float32)
    b_qkv = np.asarray(b_qkv, dtype=np.float32)
    xTs = [np.ascontiguousarray(x[b].T).astype(bf) for b in range(B)]
    in_maps = []
    for c in range(NCORES):
        b_idx, g = c // 4, c % 4
        H = [3 * g, 3 * g + 1, 3 * g + 2]
        q = lambda h: np.arange(64 * h, 64 * (h + 1))
        k = lambda h: np.arange(C + 64 * h, C + 64 * (h + 1))
        v = lambda h: np.arange(2 * C + 64 * h, 2 * C + 64 * (h + 1))
        cols = np.concatenate([
            q(H[0]), q(H[1]),
            k(H[0]), k(H[1]),
            q(H[2]), v(H[0]),
            k(H[2]), v(H[1]),
            v(H[2]),
        ])
        w_stack = w_qkv[:, cols]
        wd = w_stack.reshape(6, 128, 576).transpose(1, 0, 2).reshape(128, 6 * 576)
        b_stack = b_qkv[cols]
        bias_pad = np.zeros((128, 5), dtype=np.float32)
        for m in range(4):
            bias_pad[:, m] = b_stack[128 * m:128 * (m + 1)]
        bias_pad[:64, 4] = b_stack[512:576]
        biasa = bias_pad.copy()
        biasa[:, 0] *= A
        biasa[0:64, 2] *= A
        in_maps.append({"xT": xTs[b_idx], "w": wd.astype(bf), "biasa": biasa})
    return in_maps


def _run(x, w_qkv, b_qkv, n_head, **run_kwargs):
    assert int(n_head) == NH and x.shape == (B, T, C)
    nc = _build()
    in_maps = _prep_inputs(x, w_qkv, b_qkv)
    res = bass_utils.run_bass_kernel_spmd(
        nc, in_maps, core_ids=list(range(NCORES)), **run_kwargs)
    out = np.empty((B, T, C), dtype=np.float32)
    for c in range(NCORES):
        b_idx, g = c // 4, c % 4
        o = res.results[c]["out"].astype(np.float32)
        for i in range(3):
            num = o[65 * i:65 * i + 64]
            den = o[65 * i + 64:65 * i + 65]
            out[b_idx, :, 192 * g + 64 * i:192 * g + 64 * (i + 1)] = (num / den).T
    return out, res


def kernel(x, w_qkv, b_qkv, n_head):
    return _run(x, w_qkv, b_qkv, n_head)[0]
